# revision 31
# baseline (speedup 1.0000x reference)
"""LoFTR-style LocalFeatureTransformer (linear attention) on 8 Trainium2 cores.

Sharding: core c <-> (batch b = c//2, sequence half h = c%2). Each core holds
channel-major [256, 2400] shards of BOTH features, SBUF-resident across all
8 layers. Linear attention's KV state ([hd, hv] plus a K-sum column) is
partial-summed over the local half-sequence and AllReduced across the 2-core
pair that shares a batch.

Key structural points (v2):
- phi(x) = elu(x)+1 = exp(min(x,0)) + max(x,0).
- LayerNorm mean-subtraction is folded into the PRECEDING weight matrix:
  Wm and W2 are column-centered host-side (out = in @ (W - rowmean(W)) is
  exactly mean-centered over the output channels), so the merge/MLP matmul
  outputs need no mean matmul and no subtract.
- Variance still needs the replicated ones-matmul trick (partition-dim
  reduction); rsqrt(var+eps) = exp(-0.5*ln(var+eps)) and the attention
  1/Zden = exp(-ln(Zden)) so the ONLY ACT functions used are
  {Exp, Ln, Relu, Copy} - all resident in one activation table set
  (natural_log_exp_and_others): zero ACT_TABLE_LOADs in steady state.
- K and V projections are fused into one [128,512] moving weight per input
  channel half: one PSUM bank holds [tok,512] = [K | V] per 128-token block.
- W1's bias+ReLU is fused into single ops (ACT Relu-with-bias / DVE
  tensor_scalar) straight out of PSUM.
- Engine balance (measured on HW; the cost model's 2x/4x DVE fast modes do
  NOT materialize for dual-tensor-read ops): PE does matmuls only; ACT does
  transcendentals + half the PSUM->SBUF f16 evictions; DVE does the other
  evictions and PSUM-consuming elementwise; Pool (gpsimd) does pure-f16
  SBUF tensor_tensor ops (squares, rsd multiply, residual add). GPSIMD
  cannot touch PSUM; nothing reads 2 PSUM operands in one op; Pool has no
  scalar_tensor_tensor.
- loop2 is emitted STAGE-MAJOR: each stage for all 5 token tiles before the
  next stage, so the in-order engine queues always hold independent work
  and never head-of-line block on a cross-engine round trip (this was worth
  ~1.5x over tile-major emission). phase1's KV accumulation trails its
  projection/phi by 2 blocks for the same reason.
- The AllReduce payload is f16, pre-scaled by 1/64 on the ACT eviction (the
  scale that used to live in blockmask; exactly cancelled via Z). Per
  layer-feature, all Q/phi tiles are emitted before any Z/msg work so the
  AR hides under them; the next phase1's token blocks are interleaved into
  the producing loop2's tiles so cross-layer K/V projection (and the AR
  launch) overlaps the previous feature's MLP.
- All matmuls fp16 (fp32 PSUM accumulation).
- fast_ln2 (ln2 affine == identity, always true for this model's inputs):
  the LN2 affine collapses to dl = yd*rsd2 (DVE) + residual add (Pool);
  the general path keeps the per-channel affine on DVE.
"""

import numpy as np

import concourse.bass as bass
import concourse.mybir as mybir
import concourse.tile as tile
import bass_rust

N_CORES = 8
B, L, C = 4, 4800, 256
NHEAD, HDIM = 8, 32
R = L // 2              # tokens per core per feature: 2400
TT = 480                # channel-major token tile
NTT = R // TT           # 5
TOKEN_BLOCKS = [(i * 128, min(128, R - i * 128)) for i in range((R + 127) // 128)]
NKT = len(TOKEN_BLOCKS)  # 19
EPS_LN = 1e-5
GROUPS = [[0, 1], [2, 3], [4, 5], [6, 7]]

F32 = mybir.dt.float32
F16 = mybir.dt.float16

_ws_ctr = [0]


def split_multi_waits(nc, max_waits=1):
    """This walrus build accepts only ONE sync-wait per engine instruction.
    After TileContext exit (waits final), move excess waits onto
    EventSemaphore instructions inserted just before the owner."""
    n_split = 0
    for bb in nc.main_func.blocks:
        new_list = []
        for inst in bb.instructions:
            si = inst.sync_info
            waits = list(si.on_wait) if si is not None else []
            if len(waits) > max_waits:
                keep, extra = waits[:max_waits], waits[max_waits:]
                for w in extra:
                    _ws_ctr[0] += 1
                    ev = mybir.InstEventSemaphore(name=f"I-waitsplit-{_ws_ctr[0]}")
                    ev.engine = inst.engine
                    ev.sync_info = bass_rust.SyncInfo(on_wait=[w], on_update=[])
                    nc.register_instruction(ev)
                    new_list.append(ev)
                inst.sync_info = bass_rust.SyncInfo(
                    on_wait=keep, on_update=list(si.on_update)
                )
                n_split += 1
            new_list.append(inst)
        bb.instructions = new_list
    return n_split


def _act(nc, out, in_, func, bias=0.0, scale=1.0):
    """Raw InstActivation (bypasses the Reciprocal/Rsqrt guard; our args are
    validated end-to-end against the fp32 reference)."""
    eng = nc.scalar
    inputs = [eng.lower_ap(in_)]
    if not isinstance(bias, float):
        inputs.append(eng.lower_ap(bias))
    else:
        inputs.append(mybir.ImmediateValue(dtype=F32, value=bias))
    inputs.append(mybir.ImmediateValue(dtype=F32, value=scale))
    inputs.append(mybir.ImmediateValue(dtype=F32, value=0.0))
    return eng.add_instruction(
        mybir.InstActivation(
            name=nc.get_next_instruction_name(),
            func=func,
            ins=inputs,
            outs=[eng.lower_ap(out)],
        )
    )


def build(n_layers=8, fast_ln2=True):
    nc = bass.Bass("TRN2", target_bir_lowering=False, debug=False,
                   num_devices=N_CORES)
    AF = mybir.ActivationFunctionType
    OP = mybir.AluOpType

    xin = [nc.declare_dram_parameter(f"xT{f}", [C, R], F16, isOutput=False)
           for f in (0, 1)]
    wkv_d = nc.declare_dram_parameter("Wkv", [n_layers, C, 2 * C], F16, isOutput=False)
    wq_d = nc.declare_dram_parameter("Wq", [n_layers, C, C], F16, isOutput=False)
    wm_d = nc.declare_dram_parameter("WmC", [n_layers, C, C], F16, isOutput=False)
    w1_d = nc.declare_dram_parameter("W1", [n_layers, 2 * C, 2 * C], F16, isOutput=False)
    w2_d = nc.declare_dram_parameter("W2C", [n_layers, 2 * C, C], F16, isOutput=False)
    b1_d = nc.declare_dram_parameter("b1p", [n_layers, 128, 4], F32, isOutput=False)
    l2w_d = nc.declare_dram_parameter("l2wp", [n_layers, 128, 2], F32, isOutput=False)
    l2b_d = nc.declare_dram_parameter("l2bp", [n_layers, 128, 2], F32, isOutput=False)
    mask_d = nc.declare_dram_parameter("blockmask", [128, 128], F16, isOutput=False)
    ones_d = nc.declare_dram_parameter("onesC", [128, 128], F16, isOutput=False)
    yout = [nc.declare_dram_parameter(f"yT{f}", [C, R], F16, isOutput=True)
            for f in (0, 1)]

    with tile.TileContext(nc) as tc:
        with (
            tc.tile_pool(name="const", bufs=1) as constp,
            tc.tile_pool(name="feat", bufs=1) as featp,
            tc.tile_pool(name="wpool", bufs=2) as wp,
            tc.tile_pool(name="callp", bufs=2) as callp,
            tc.tile_pool(name="p1s", bufs=4) as p1s,
            tc.tile_pool(name="p2s", bufs=3) as p2s,
            tc.tile_pool(name="dramp", bufs=2, space="DRAM") as dramp,
            tc.tile_pool(name="psump", bufs=1, space="PSUM") as psump,
        ):
            mask = constp.tile([128, 128], F16, tag="mask", name="mask")
            nc.sync.dma_start(out=mask[:], in_=mask_d[:])
            ones = constp.tile([128, 128], F16, tag="ones", name="ones")
            nc.sync.dma_start(out=ones[:], in_=ones_d[:])
            epsln = constp.tile([128, 1], F32, tag="epsln", name="epsln")
            nc.vector.memset(epsln[:], EPS_LN)

            x = {}
            for f in (0, 1):
                for ci in (0, 1):
                    t = featp.tile([128, R], F16, tag=f"x{f}{ci}", name=f"x{f}{ci}")
                    nc.sync.dma_start(out=t[:], in_=xin[f][ci * 128:(ci + 1) * 128, :])
                    x[(f, ci)] = t

            class Phase1:
                """K/V projection + phi + KV accumulation, block-streamable."""

                def __init__(self, src, w):
                    self.src, self.w = src, w
                    self.kvps = [psump.tile([128, 258], F32, tag="kv", bufs=2,
                                            name=f"kvps{mo}") for mo in (0, 1)]
                    self.done = 0
                    self.staged = []

                def _proj(self, kt):
                    """K|V projection + phi for one token block."""
                    src, w = self.src, self.w
                    t0, tn = TOKEN_BLOCKS[kt]
                    pkv = psump.tile([128, 512], F32, tag="ring", bufs=6,
                                     name="pkv")
                    for ci in (0, 1):
                        nc.tensor.matmul(pkv[:tn, :],
                                         x[(src, ci)][:, t0:t0 + tn],
                                         w["wkv"][ci][:],
                                         start=(ci == 0), stop=(ci == 1))
                    kv16 = p1s.tile([128, 514], F16, tag="kv16", bufs=6,
                                    name="kv16")
                    # evict K|V; alternate engines to balance load
                    if kt % 2 == 0:
                        nc.scalar.copy(kv16[:tn, 0:512], pkv[:tn, :])
                    else:
                        nc.vector.tensor_copy(kv16[:tn, 0:512], pkv[:tn, :])
                    nc.gpsimd.memset(kv16[:tn, 512:514], 1.0)
                    m1 = p1s.tile([128, 256], F16, tag="m1", bufs=4, name="m1")
                    nc.vector.tensor_scalar_min(m1[:tn, :], kv16[:tn, 0:256], 0.0)
                    _act(nc, m1[:tn, :], m1[:tn, :], AF.Exp)
                    ktok = p1s.tile([128, 256], F16, tag="ktok", bufs=6,
                                    name="ktok")
                    nc.vector.scalar_tensor_tensor(
                        ktok[:tn, :], kv16[:tn, 0:256], 0.0, m1[:tn, :],
                        OP.max, OP.add)
                    self.staged.append((kt, tn, ktok, kv16))

                def _accum(self):
                    kt, tn, ktok, kv16 = self.staged.pop(0)
                    for mo in (0, 1):
                        nc.tensor.matmul(self.kvps[mo][:, :],
                                         ktok[:tn, mo * 128:(mo + 1) * 128],
                                         kv16[:tn, 256:514],
                                         start=(kt == 0), stop=(kt == NKT - 1))

                def blocks(self, upto):
                    """Software-pipelined: KV accumulation trails the
                    projection/phi by 2 blocks so the PE never head-of-line
                    stalls on the phi round-trip."""
                    upto = min(upto, NKT)
                    for kt in range(self.done, upto):
                        self._proj(kt)
                        if len(self.staged) > 2:
                            self._accum()
                    if upto == NKT:
                        while self.staged:
                            self._accum()
                    self.done = max(self.done, upto)

                def finish(self):
                    self.blocks(NKT)
                    # f16 AR payload, pre-scaled by 1/64 (the scale that used
                    # to live in blockmask) so |KV| and |Ksum| fit f16 range.
                    # The post-AR state is cast to f16 anyway, so this only
                    # moves the rounding before the pairwise add.
                    arin = dramp.tile([2, 128, 257], F16, tag="arin", name="arin")
                    arout = dramp.tile([2, 128, 257], F16, tag="arout", name="arout")
                    for mo in (0, 1):
                        t = callp.tile([128, 257], F16, tag=f"kvsb{mo}",
                                       name=f"kvsb{mo}")
                        _act(nc, t[:], self.kvps[mo][:, 0:257], AF.Copy,
                             scale=1.0 / 64.0)
                        nc.sync.dma_start(out=arin[mo], in_=t[:])
                    nc.gpsimd.collective_compute(
                        "AllReduce", OP.add, replica_groups=GROUPS,
                        ins=[arin.opt()], outs=[arout.opt()])
                    return arout

            def finish_kv(arout):
                """Pull the AllReduced KV state back; mask + build Ksum_bcast."""
                kvbd, ksb = [], []
                for ci in (0, 1):
                    t = callp.tile([128, 257], F16, tag=f"kvar{ci}", name=f"kvar{ci}")
                    nc.sync.dma_start(out=t[:], in_=arout[ci])
                    bd = callp.tile([128, 128], F16, tag=f"kvbd{ci}", name=f"kvbd{ci}")
                    nc.vector.tensor_tensor(bd[:], t[:, ci * 128:(ci + 1) * 128],
                                            mask[:], OP.mult)
                    ks32 = callp.tile([128, 1], F32, tag=f"ks32{ci}",
                                      name=f"ks32{ci}")
                    nc.vector.tensor_copy(ks32[:], t[:, 256:257])
                    kb = callp.tile([128, 128], F16, tag=f"ksb{ci}", name=f"ksb{ci}")
                    nc.vector.tensor_scalar(kb[:], mask[:], ks32[:], None,
                                            OP.mult)
                    kvbd.append(bd)
                    ksb.append(kb)
                return kvbd, ksb

            class QPhi:
                """Q projection + phi, tile-streamable (AR-independent).
                Emitted either standalone or interleaved into a running
                loop2 via its per-tile callback, keeping the PE fed across
                the loop boundary."""

                def __init__(self, f, w):
                    self.f, self.w = f, w
                    self.qphi_all = []
                    self.done = 0

                def emit(self, upto):
                    f, w = self.f, self.w
                    for it in range(self.done, min(upto, NTT)):
                        ts = slice(it * TT, it * TT + TT)
                        pq_ci, qp_ci = [], []
                        for ci in (0, 1):
                            pq = psump.tile([128, TT], F32, tag="ring", bufs=6,
                                            name="pq")
                            for cj in (0, 1):
                                nc.tensor.matmul(
                                    pq[:],
                                    w["wq"][cj][:, ci * 128:(ci + 1) * 128],
                                    x[(f, cj)][:, ts],
                                    start=(cj == 0), stop=(cj == 1))
                            pq_ci.append(pq)
                        for ci in (0, 1):
                            mq = p2s.tile([128, TT], F16, tag=f"mq{ci}", bufs=4,
                                          name="mq")
                            nc.vector.tensor_scalar_min(mq[:], pq_ci[ci][:], 0.0)
                            _act(nc, mq[:], mq[:], AF.Exp)
                            qp = p2s.tile([128, TT], F16, tag=f"qphi{ci}",
                                          bufs=7, name="qp")
                            nc.vector.scalar_tensor_tensor(
                                qp[:], pq_ci[ci][:], 0.0, mq[:], OP.max, OP.add)
                            qp_ci.append(qp)
                        self.qphi_all.append(qp_ci)
                    self.done = max(self.done, min(upto, NTT))

                def finish(self):
                    self.emit(NTT)
                    return self.qphi_all

            def loop2(f, w, kvbd, ksb, qphi_all, after_tile=None):
                """Z, msg, merge, LN1(var-only), MLP, LN2(var-only), residual.
                STAGE-MAJOR: each stage is emitted for all NTT tiles before
                the next stage, so every in-order engine queue always holds
                independent work from other tiles and never head-of-line
                blocks on a cross-engine round trip."""
                tss = [slice(it * TT, it * TT + TT) for it in range(NTT)]
                # S1: Zden matmuls + reciprocal via exp(-ln)
                zr_all = []
                for it in range(NTT):
                    zr = []
                    for ci in (0, 1):
                        pz = psump.tile([128, TT], F32, tag="ring", bufs=6,
                                        name="pz")
                        nc.tensor.matmul(pz[:], ksb[ci][:], qphi_all[it][ci][:],
                                         start=True, stop=True)
                        zl = p2s.tile([128, TT], F32, tag=f"zl{ci}", bufs=2, name="zl")
                        _act(nc, zl[:], pz[:], AF.Ln)
                        z = p2s.tile([128, TT], F16, tag=f"zr{ci}", bufs=6,
                                     name="zr")
                        _act(nc, z[:], zl[:], AF.Exp, scale=-1.0)
                        zr.append(z)
                    zr_all.append(zr)
                # S2: msg matmuls + Z scaling
                msgz_all = []
                for it in range(NTT):
                    msgz = []
                    for ci in (0, 1):
                        pm = psump.tile([128, TT], F32, tag="ring", bufs=6,
                                        name="pm")
                        nc.tensor.matmul(pm[:], kvbd[ci][:], qphi_all[it][ci][:],
                                         start=True, stop=True)
                        mz = p2s.tile([128, TT], F16, tag=f"msgz{ci}", bufs=6,
                                      name="mz")
                        nc.vector.tensor_tensor(mz[:], pm[:], zr_all[it][ci][:],
                                                OP.mult)
                        msgz.append(mz)
                    msgz_all.append(msgz)
                # S3: merge (centered Wm) + evict + square
                yc_all, y2_all = [], []
                for it in range(NTT):
                    yc, y2 = [], []
                    for mo in (0, 1):
                        pmg = psump.tile([128, TT], F32, tag="ring", bufs=6,
                                         name="pmg")
                        for ci in (0, 1):
                            nc.tensor.matmul(pmg[:],
                                             w["wm"][ci][:, mo * 128:(mo + 1) * 128],
                                             msgz_all[it][ci][:],
                                             start=(ci == 0), stop=(ci == 1))
                        t = p2s.tile([128, TT], F16, tag=f"yc{mo}", bufs=6,
                                     name="yc")
                        nc.scalar.copy(t[:], pmg[:])
                        yc.append(t)
                        t2 = p2s.tile([128, TT], F16, tag=f"y2{mo}", bufs=3,
                                      name="y2")
                        nc.gpsimd.tensor_tensor(t2[:], t[:], t[:], OP.mult)
                        y2.append(t2)
                    yc_all.append(yc)
                    y2_all.append(y2)
                # S4: var matmul + rsqrt via exp(-0.5 ln)
                rsd_all = []
                for it in range(NTT):
                    pvar = psump.tile([128, TT], F32, tag="ring", bufs=6,
                                      name="pvar")
                    for mo in (0, 1):
                        nc.tensor.matmul(pvar[:], ones[:], y2_all[it][mo][:],
                                         start=(mo == 0), stop=(mo == 1))
                    tv = p2s.tile([128, TT], F32, tag="tv", bufs=2, name="tv")
                    _act(nc, tv[:], pvar[:], AF.Ln, bias=epsln[:, 0:1])
                    rsd = p2s.tile([128, TT], F16, tag="rsd", bufs=6, name="rsd")
                    _act(nc, rsd[:], tv[:], AF.Exp, scale=-0.5)
                    rsd_all.append(rsd)
                # S5: msghat (f16 x f16 on Pool; DVE is the scarcer engine)
                mh_all = []
                for it in range(NTT):
                    mh = []
                    for mo in (0, 1):
                        t = p2s.tile([128, TT], F16, tag=f"mh{mo}", bufs=6,
                                     name="mh")
                        nc.gpsimd.tensor_tensor(t[:], yc_all[it][mo][:],
                                                rsd_all[it][:], OP.mult)
                        mh.append(t)
                    mh_all.append(mh)
                # S6: W1 + fused bias+relu
                r1_all = []
                for it in range(NTT):
                    r1 = []
                    for mo in range(4):
                        pw1 = psump.tile([128, TT], F32, tag="ring", bufs=6,
                                         name="pw1")
                        for cj in range(4):
                            rhs = (x[(f, cj)][:, tss[it]] if cj < 2
                                   else mh_all[it][cj - 2][:])
                            nc.tensor.matmul(pw1[:],
                                             w["w1"][cj][:, mo * 128:(mo + 1) * 128],
                                             rhs, start=(cj == 0), stop=(cj == 3))
                        t = p2s.tile([128, TT], F16, tag=f"r1{mo}", bufs=6,
                                     name="r1")
                        if mo < 2:
                            _act(nc, t[:], pw1[:], AF.Relu,
                                 bias=w["b1"][:, mo:mo + 1])
                        else:
                            nc.vector.tensor_scalar(t[:], pw1[:],
                                                    w["b1"][:, mo:mo + 1], 0.0,
                                                    OP.add, OP.max)
                        r1.append(t)
                    r1_all.append(r1)
                # S7: W2 (centered) + evict + square
                yd_all, yd2_all = [], []
                for it in range(NTT):
                    yd, yd2 = [], []
                    for mo in (0, 1):
                        pw2 = psump.tile([128, TT], F32, tag="ring", bufs=6,
                                         name="pw2")
                        for cj in range(4):
                            nc.tensor.matmul(pw2[:],
                                             w["w2"][cj][:, mo * 128:(mo + 1) * 128],
                                             r1_all[it][cj][:], start=(cj == 0),
                                             stop=(cj == 3))
                        t = p2s.tile([128, TT], F16, tag=f"yd{mo}", bufs=6,
                                     name="yd")
                        nc.vector.tensor_copy(t[:], pw2[:])
                        yd.append(t)
                        t2 = p2s.tile([128, TT], F16, tag=f"yd2{mo}", bufs=3,
                                      name="yd2")
                        nc.gpsimd.tensor_tensor(t2[:], t[:], t[:], OP.mult)
                        yd2.append(t2)
                    yd_all.append(yd)
                    yd2_all.append(yd2)
                # S8: var2 + rsqrt
                rsd2_all = []
                for it in range(NTT):
                    pvar2 = psump.tile([128, TT], F32, tag="ring", bufs=6,
                                       name="pvar2")
                    for mo in (0, 1):
                        nc.tensor.matmul(pvar2[:], ones[:], yd2_all[it][mo][:],
                                         start=(mo == 0), stop=(mo == 1))
                    tv2 = p2s.tile([128, TT], F32, tag="tv2", bufs=2, name="tv2")
                    _act(nc, tv2[:], pvar2[:], AF.Ln, bias=epsln[:, 0:1])
                    rsd2 = p2s.tile([128, TT], F16, tag="rsd2", bufs=6,
                                    name="rsd2")
                    _act(nc, rsd2[:], tv2[:], AF.Exp, scale=-0.5)
                    rsd2_all.append(rsd2)
                # S9: LN2 affine + residual update. fast_ln2 (ln2 affine is
                # identity, true for this model): dl = yd*rsd2 on DVE, the
                # residual add on Pool.
                for it in range(NTT):
                    for mo in (0, 1):
                        dl = p2s.tile([128, TT], F16, tag=f"dl{mo}", bufs=2, name="dl")
                        if fast_ln2:
                            nc.vector.tensor_tensor(
                                dl[:], yd_all[it][mo][:], rsd2_all[it][:],
                                OP.mult)
                            nc.gpsimd.tensor_tensor(
                                x[(f, mo)][:, tss[it]], x[(f, mo)][:, tss[it]],
                                dl[:], OP.add)
                        else:
                            nc.vector.scalar_tensor_tensor(
                                dl[:], yd_all[it][mo][:], w["l2w"][:, mo:mo + 1],
                                rsd2_all[it][:], OP.mult, OP.mult)
                            nc.vector.scalar_tensor_tensor(
                                x[(f, mo)][:, tss[it]], dl[:],
                                w["l2b"][:, mo:mo + 1],
                                x[(f, mo)][:, tss[it]], OP.add, OP.add)
                    if after_tile is not None:
                        after_tile(it)

            def load_weights(li):
                w = {}
                w["wkv"] = []
                for ci in (0, 1):
                    t = wp.tile([128, 512], F16, tag=f"wkv{ci}", name=f"wkv{ci}")
                    nc.sync.dma_start(
                        out=t[:], in_=wkv_d[li, ci * 128:(ci + 1) * 128, :])
                    w["wkv"].append(t)
                for nm, dram in (("wq", wq_d), ("wm", wm_d)):
                    tiles = []
                    for ci in (0, 1):
                        t = wp.tile([128, 256], F16, tag=f"{nm}{ci}",
                                    name=f"{nm}{ci}")
                        nc.sync.dma_start(
                            out=t[:], in_=dram[li, ci * 128:(ci + 1) * 128, :])
                        tiles.append(t)
                    w[nm] = tiles
                w["w1"] = []
                for ci in range(4):
                    t = wp.tile([128, 512], F16, tag=f"w1{ci}", name=f"w1{ci}")
                    nc.sync.dma_start(
                        out=t[:], in_=w1_d[li, ci * 128:(ci + 1) * 128, :])
                    w["w1"].append(t)
                w["w2"] = []
                for ci in range(4):
                    t = wp.tile([128, 256], F16, tag=f"w2{ci}", name=f"w2{ci}")
                    nc.sync.dma_start(
                        out=t[:], in_=w2_d[li, ci * 128:(ci + 1) * 128, :])
                    w["w2"].append(t)
                for nm, dram, nf in (("b1", b1_d, 4), ("l2w", l2w_d, 2),
                                     ("l2b", l2b_d, 2)):
                    t = wp.tile([128, nf], F32, tag=nm, name=nm)
                    nc.sync.dma_start(out=t[:], in_=dram[li])
                    w[nm] = t
                return w

            # per-tile callback: stream pending Phase1 token blocks (paced by
            # the x tiles they read) and pending QPhi tiles into the running
            # loop2 so the PE stays fed across loop/layer boundaries.
            def interleave_cb(p1, qp):
                if p1 is None and qp is None:
                    return None
                def cb(it):
                    if p1 is not None:
                        p1.blocks(int((it + 1) * TT / 128))
                    if qp is not None:
                        qp.emit(it + 1)
                return cb

            # schedule
            LAYER_NAMES = ['self', 'cross'] * (n_layers // 2 + 1)
            ws = [load_weights(0)]
            pending = None   # Phase1 for the next consumer, streamed early
            pend_qp = None   # QPhi for feature 0 of the next layer

            for li in range(n_layers):
                w = ws[li]
                if li + 1 < n_layers:
                    ws.append(load_weights(li + 1))
                nxt = ws[li + 1] if li + 1 < n_layers else None
                if LAYER_NAMES[li] == 'self':
                    if pending is None:
                        p1a = Phase1(0, w)
                        p1a.blocks(NKT)
                    else:
                        p1a = pending
                    ar0 = p1a.finish()
                    p1b = Phase1(1, w)
                    p1b.blocks(NKT)
                    ar1 = p1b.finish()
                    qp0 = pend_qp if pend_qp is not None else QPhi(0, w)
                    qphi0 = qp0.finish()
                    kvbd0, ksb0 = finish_kv(ar0)
                    qp1 = QPhi(1, w)
                    loop2(0, w, kvbd0, ksb0, qphi0)
                    qphi1 = qp1.finish()
                    kvbd1, ksb1 = finish_kv(ar1)
                    # next layer is cross: P1(f0, src=f1) + QPhi(f0, next)
                    pending = Phase1(1, nxt) if nxt is not None else None
                    pend_qp = None
                    loop2(1, w, kvbd1, ksb1, qphi1,
                          interleave_cb(pending, None))
                else:
                    if pending is None:
                        p1a = Phase1(1, w)
                        p1a.blocks(NKT)
                    else:
                        p1a = pending
                    ar0 = p1a.finish()
                    qp0 = pend_qp if pend_qp is not None else QPhi(0, w)
                    qphi0 = qp0.finish()
                    kvbd0, ksb0 = finish_kv(ar0)
                    # P1(f1, src=f0) + QPhi(f1, this layer) stream off loop2(0)
                    p1b = Phase1(0, w)
                    qp1 = QPhi(1, w)
                    loop2(0, w, kvbd0, ksb0, qphi0, interleave_cb(p1b, None))
                    ar1 = p1b.finish()
                    qphi1 = qp1.finish()
                    kvbd1, ksb1 = finish_kv(ar1)
                    # next layer is self: P1(f0, src=f0) + QPhi(f0, next)
                    pending = Phase1(0, nxt) if nxt is not None else None
                    pend_qp = None
                    loop2(1, w, kvbd1, ksb1, qphi1,
                          interleave_cb(pending, None))

            for f in (0, 1):
                for ci in (0, 1):
                    nc.sync.dma_start(out=yout[f][ci * 128:(ci + 1) * 128, :],
                                      in_=x[(f, ci)][:])

    split_multi_waits(nc)
    return nc


def prep_inputs(inputs, n_layers=8):
    """Host-side: shard features, fold ln1 into W1/bias1, column-center
    Wm and W2 (folds the LayerNorm mean-subtract into the matmul), pack
    constants. Returns in_maps for the 8 cores."""
    f32 = np.float32
    feat0, feat1 = np.asarray(inputs["feat0"]), np.asarray(inputs["feat1"])
    Wq, Wk, Wv, Wm = (np.asarray(inputs[k], dtype=f32)
                      for k in ("Wq", "Wk", "Wv", "Wm"))
    W1, W2 = np.asarray(inputs["W1"], dtype=f32), np.asarray(inputs["W2"], dtype=f32)
    ln1_w, ln1_b = np.asarray(inputs["ln1_w"], dtype=f32), np.asarray(inputs["ln1_b"], dtype=f32)
    ln2_w, ln2_b = np.asarray(inputs["ln2_w"], dtype=f32), np.asarray(inputs["ln2_b"], dtype=f32)

    W1eff = W1[:n_layers].copy()
    W1eff[:, C:, :] *= ln1_w[:n_layers, :, None]
    b1 = np.einsum("lc,lcd->ld", ln1_b[:n_layers], W1[:n_layers, C:, :])
    b1p = np.ascontiguousarray(b1.reshape(n_layers, 4, 128).transpose(0, 2, 1))
    l2wp = np.ascontiguousarray(ln2_w[:n_layers].reshape(n_layers, 2, 128).transpose(0, 2, 1))
    l2bp = np.ascontiguousarray(ln2_b[:n_layers].reshape(n_layers, 2, 128).transpose(0, 2, 1))

    # column-center Wm and W2: out @ (W - rowmean) is exactly mean-centered
    WmC = Wm[:n_layers] - Wm[:n_layers].mean(axis=2, keepdims=True)
    W2C = W2[:n_layers] - W2[:n_layers].mean(axis=2, keepdims=True)
    # fused K|V projection weight [C, 2C] = [Wk | Wv]
    Wkv = np.concatenate([Wk[:n_layers], Wv[:n_layers]], axis=2)

    f16 = np.float16
    idx = np.arange(128)
    # plain 0/1 mask: the 1/64 range scale is applied before the AllReduce
    blockmask = (idx[:, None] // 32 == idx[None, :] // 32).astype(f16)
    onesC = np.full((128, 128), 1.0 / C, dtype=f16)

    shared = {
        "Wkv": np.ascontiguousarray(Wkv).astype(f16),
        "Wq": np.ascontiguousarray(Wq[:n_layers]).astype(f16),
        "WmC": np.ascontiguousarray(WmC).astype(f16),
        "W1": np.ascontiguousarray(W1eff).astype(f16),
        "W2C": np.ascontiguousarray(W2C).astype(f16),
        "b1p": b1p, "l2wp": l2wp, "l2bp": l2bp,
        "blockmask": blockmask, "onesC": onesC,
    }
    in_maps = []
    for c in range(N_CORES):
        b, h = c // 2, c % 2
        rows = slice(h * R, (h + 1) * R)
        m = dict(shared)
        m["xT0"] = np.ascontiguousarray(feat0[b, rows].T).astype(f16)
        m["xT1"] = np.ascontiguousarray(feat1[b, rows].T).astype(f16)
        in_maps.append(m)
    return in_maps


def assemble_outputs(results):
    feat0 = np.empty((B, L, C), np.float32)
    feat1 = np.empty((B, L, C), np.float32)
    for c in range(N_CORES):
        b, h = c // 2, c % 2
        rows = slice(h * R, (h + 1) * R)
        feat0[b, rows] = results[c]["yT0"].T.astype(np.float32)
        feat1[b, rows] = results[c]["yT1"].T.astype(np.float32)
    return feat0, feat1


_cache = {}


def get_nc(n_layers=8, fast_ln2=True):
    key = (n_layers, fast_ln2)
    if key not in _cache:
        _cache[key] = build(n_layers, fast_ln2)
    return _cache[key]


def kernel(**inputs):
    from concourse.bass_utils import run_bass_kernel_spmd
    fast = (np.all(np.asarray(inputs["ln2_w"]) == 1.0)
            and np.all(np.asarray(inputs["ln2_b"]) == 0.0))
    nc = get_nc(8, fast)
    in_maps = prep_inputs(inputs, 8)
    res = run_bass_kernel_spmd(nc, in_maps, list(range(N_CORES)))
    return assemble_outputs(res.results)


# revision 32
# speedup vs baseline: 1.0589x; 1.0589x over previous
"""LoFTR-style LocalFeatureTransformer (linear attention) on 8 Trainium2 cores.

Sharding: core c <-> (batch b = c//2, sequence half h = c%2). Each core holds
channel-major [256, 2400] shards of BOTH features, SBUF-resident across all
8 layers. Linear attention's KV state ([hd, hv] plus a K-sum column) is
partial-summed over the local half-sequence and AllReduced across the 2-core
pair that shares a batch.

Key structural points (v2):
- phi(x) = elu(x)+1 = exp(min(x,0)) + max(x,0).
- LayerNorm mean-subtraction is folded into the PRECEDING weight matrix:
  Wm and W2 are column-centered host-side (out = in @ (W - rowmean(W)) is
  exactly mean-centered over the output channels), so the merge/MLP matmul
  outputs need no mean matmul and no subtract.
- Variance still needs the replicated ones-matmul trick (partition-dim
  reduction); rsqrt(var+eps) = exp(-0.5*ln(var+eps)) and the attention
  1/Zden = exp(-ln(Zden)) so the ONLY ACT functions used are
  {Exp, Ln, Relu, Copy} - all resident in one activation table set
  (natural_log_exp_and_others): zero ACT_TABLE_LOADs in steady state.
- K and V projections are fused into one [128,512] moving weight per input
  channel half: one PSUM bank holds [tok,512] = [K | V] per 128-token block.
- W1's bias+ReLU is fused into single ops (ACT Relu-with-bias / DVE
  tensor_scalar) straight out of PSUM.
- Engine balance (measured on HW; the cost model's 2x/4x DVE fast modes do
  NOT materialize for dual-tensor-read ops): PE does matmuls only; ACT does
  transcendentals + half the PSUM->SBUF f16 evictions; DVE does the other
  evictions and PSUM-consuming elementwise; Pool (gpsimd) does pure-f16
  SBUF tensor_tensor ops (squares, rsd multiply, residual add). GPSIMD
  cannot touch PSUM; nothing reads 2 PSUM operands in one op; Pool has no
  scalar_tensor_tensor.
- loop2 is emitted STAGE-MAJOR: each stage for all 5 token tiles before the
  next stage, so the in-order engine queues always hold independent work
  and never head-of-line block on a cross-engine round trip (this was worth
  ~1.5x over tile-major emission). phase1's KV accumulation trails its
  projection/phi by 2 blocks for the same reason.
- The AllReduce payload is f16, pre-scaled by 1/64 on the ACT eviction (the
  scale that used to live in blockmask; exactly cancelled via Z). Per
  layer-feature, all Q/phi tiles are emitted before any Z/msg work so the
  AR hides under them; the next phase1's token blocks are interleaved into
  the producing loop2's tiles so cross-layer K/V projection (and the AR
  launch) overlaps the previous feature's MLP.
- All matmuls fp16 (fp32 PSUM accumulation).
- fast_ln2 (ln2 affine == identity, always true for this model's inputs):
  the LN2 affine collapses to dl = yd*rsd2 (DVE) + residual add (Pool);
  the general path keeps the per-channel affine on DVE.
"""

import numpy as np

import concourse.bass as bass
import concourse.mybir as mybir
import concourse.tile as tile
import bass_rust

N_CORES = 8
B, L, C = 4, 4800, 256
NHEAD, HDIM = 8, 32
R = L // 2              # tokens per core per feature: 2400
TT = 480                # channel-major token tile
NTT = R // TT           # 5
TOKEN_BLOCKS = [(i * 128, min(128, R - i * 128)) for i in range((R + 127) // 128)]
NKT = len(TOKEN_BLOCKS)  # 19
EPS_LN = 1e-5
GROUPS = [[0, 1], [2, 3], [4, 5], [6, 7]]

F32 = mybir.dt.float32
F16 = mybir.dt.float16

_ws_ctr = [0]


def split_multi_waits(nc, max_waits=1):
    """This walrus build accepts only ONE sync-wait per engine instruction.
    After TileContext exit (waits final), move excess waits onto
    EventSemaphore instructions inserted just before the owner."""
    n_split = 0
    for bb in nc.main_func.blocks:
        new_list = []
        for inst in bb.instructions:
            si = inst.sync_info
            waits = list(si.on_wait) if si is not None else []
            if len(waits) > max_waits:
                keep, extra = waits[:max_waits], waits[max_waits:]
                for w in extra:
                    _ws_ctr[0] += 1
                    ev = mybir.InstEventSemaphore(name=f"I-waitsplit-{_ws_ctr[0]}")
                    ev.engine = inst.engine
                    ev.sync_info = bass_rust.SyncInfo(on_wait=[w], on_update=[])
                    nc.register_instruction(ev)
                    new_list.append(ev)
                inst.sync_info = bass_rust.SyncInfo(
                    on_wait=keep, on_update=list(si.on_update)
                )
                n_split += 1
            new_list.append(inst)
        bb.instructions = new_list
    return n_split


def _act(nc, out, in_, func, bias=0.0, scale=1.0):
    """Raw InstActivation (bypasses the Reciprocal/Rsqrt guard; our args are
    validated end-to-end against the fp32 reference)."""
    eng = nc.scalar
    inputs = [eng.lower_ap(in_)]
    if not isinstance(bias, float):
        inputs.append(eng.lower_ap(bias))
    else:
        inputs.append(mybir.ImmediateValue(dtype=F32, value=bias))
    inputs.append(mybir.ImmediateValue(dtype=F32, value=scale))
    inputs.append(mybir.ImmediateValue(dtype=F32, value=0.0))
    return eng.add_instruction(
        mybir.InstActivation(
            name=nc.get_next_instruction_name(),
            func=func,
            ins=inputs,
            outs=[eng.lower_ap(out)],
        )
    )


def build(n_layers=8, fast_ln2=True):
    nc = bass.Bass("TRN2", target_bir_lowering=False, debug=False,
                   num_devices=N_CORES)
    AF = mybir.ActivationFunctionType
    OP = mybir.AluOpType

    xin = [nc.declare_dram_parameter(f"xT{f}", [C, R], F16, isOutput=False)
           for f in (0, 1)]
    wkv_d = nc.declare_dram_parameter("Wkv", [n_layers, C, 2 * C], F16, isOutput=False)
    wq_d = nc.declare_dram_parameter("Wq", [n_layers, C, C], F16, isOutput=False)
    wm_d = nc.declare_dram_parameter("WmC", [n_layers, C, C], F16, isOutput=False)
    w1_d = nc.declare_dram_parameter("W1", [n_layers, 2 * C, 2 * C], F16, isOutput=False)
    w2_d = nc.declare_dram_parameter("W2C", [n_layers, 2 * C, C], F16, isOutput=False)
    b1_d = nc.declare_dram_parameter("b1p", [n_layers, 128, 4], F32, isOutput=False)
    l2w_d = nc.declare_dram_parameter("l2wp", [n_layers, 128, 2], F32, isOutput=False)
    l2b_d = nc.declare_dram_parameter("l2bp", [n_layers, 128, 2], F32, isOutput=False)
    mask_d = nc.declare_dram_parameter("blockmask", [128, 128], F16, isOutput=False)
    ones_d = nc.declare_dram_parameter("onesC", [128, 128], F16, isOutput=False)
    yout = [nc.declare_dram_parameter(f"yT{f}", [C, R], F16, isOutput=True)
            for f in (0, 1)]

    with tile.TileContext(nc) as tc:
        with (
            tc.tile_pool(name="const", bufs=1) as constp,
            tc.tile_pool(name="feat", bufs=1) as featp,
            tc.tile_pool(name="wpool", bufs=2) as wp,
            tc.tile_pool(name="callp", bufs=2) as callp,
            tc.tile_pool(name="p1s", bufs=4) as p1s,
            tc.tile_pool(name="p2s", bufs=3) as p2s,
            tc.tile_pool(name="dramp", bufs=2, space="DRAM") as dramp,
            tc.tile_pool(name="psump", bufs=1, space="PSUM") as psump,
        ):
            mask = constp.tile([128, 128], F16, tag="mask", name="mask")
            nc.sync.dma_start(out=mask[:], in_=mask_d[:])
            ones = constp.tile([128, 128], F16, tag="ones", name="ones")
            nc.sync.dma_start(out=ones[:], in_=ones_d[:])
            epsln = constp.tile([128, 1], F32, tag="epsln", name="epsln")
            nc.vector.memset(epsln[:], EPS_LN)

            x = {}
            for f in (0, 1):
                for ci in (0, 1):
                    t = featp.tile([128, R], F16, tag=f"x{f}{ci}", name=f"x{f}{ci}")
                    nc.sync.dma_start(out=t[:], in_=xin[f][ci * 128:(ci + 1) * 128, :])
                    x[(f, ci)] = t

            class Phase1:
                """K/V projection + phi + KV accumulation, block-streamable."""

                def __init__(self, src, w):
                    self.src, self.w = src, w
                    self.kvps = [psump.tile([128, 258], F32, tag="kv", bufs=2,
                                            name=f"kvps{mo}") for mo in (0, 1)]
                    self.done = 0
                    self.staged = []

                def _proj(self, kt):
                    """K|V projection + phi for one token block."""
                    src, w = self.src, self.w
                    t0, tn = TOKEN_BLOCKS[kt]
                    pkv = psump.tile([128, 512], F32, tag="ring", bufs=6,
                                     name="pkv")
                    for ci in (0, 1):
                        nc.tensor.matmul(pkv[:tn, :],
                                         x[(src, ci)][:, t0:t0 + tn],
                                         w["wkv"][ci][:],
                                         start=(ci == 0), stop=(ci == 1))
                    kv16 = p1s.tile([128, 514], F16, tag="kv16", bufs=6,
                                    name="kv16")
                    # evict K|V; alternate engines to balance load
                    if kt % 2 == 0:
                        nc.scalar.copy(kv16[:tn, 0:512], pkv[:tn, :])
                    else:
                        nc.vector.tensor_copy(kv16[:tn, 0:512], pkv[:tn, :])
                    nc.gpsimd.memset(kv16[:tn, 512:514], 1.0)
                    m1 = p1s.tile([128, 256], F16, tag="m1", bufs=4, name="m1")
                    nc.vector.tensor_scalar_min(m1[:tn, :], kv16[:tn, 0:256], 0.0)
                    _act(nc, m1[:tn, :], m1[:tn, :], AF.Exp)
                    ktok = p1s.tile([128, 256], F16, tag="ktok", bufs=6,
                                    name="ktok")
                    nc.vector.scalar_tensor_tensor(
                        ktok[:tn, :], kv16[:tn, 0:256], 0.0, m1[:tn, :],
                        OP.max, OP.add)
                    self.staged.append((kt, tn, ktok, kv16))

                def _accum(self):
                    kt, tn, ktok, kv16 = self.staged.pop(0)
                    for mo in (0, 1):
                        nc.tensor.matmul(self.kvps[mo][:, :],
                                         ktok[:tn, mo * 128:(mo + 1) * 128],
                                         kv16[:tn, 256:514],
                                         start=(kt == 0), stop=(kt == NKT - 1))

                def blocks(self, upto):
                    """Software-pipelined: KV accumulation trails the
                    projection/phi by 2 blocks so the PE never head-of-line
                    stalls on the phi round-trip."""
                    upto = min(upto, NKT)
                    for kt in range(self.done, upto):
                        self._proj(kt)
                        if len(self.staged) > 2:
                            self._accum()
                    if upto == NKT:
                        while self.staged:
                            self._accum()
                    self.done = max(self.done, upto)

                def finish(self):
                    self.blocks(NKT)
                    # f16 AR payload, pre-scaled by 1/64 (the scale that used
                    # to live in blockmask) so |KV| and |Ksum| fit f16 range.
                    # The post-AR state is cast to f16 anyway, so this only
                    # moves the rounding before the pairwise add.
                    arin = dramp.tile([2, 128, 257], F16, tag="arin", name="arin")
                    arout = dramp.tile([2, 128, 257], F16, tag="arout", name="arout")
                    for mo in (0, 1):
                        t = callp.tile([128, 257], F16, tag=f"kvsb{mo}",
                                       name=f"kvsb{mo}")
                        _act(nc, t[:], self.kvps[mo][:, 0:257], AF.Copy,
                             scale=1.0 / 64.0)
                        nc.sync.dma_start(out=arin[mo], in_=t[:])
                    nc.gpsimd.collective_compute(
                        "AllReduce", OP.add, replica_groups=GROUPS,
                        ins=[arin.opt()], outs=[arout.opt()])
                    return arout

            def finish_kv(arout):
                """Pull the AllReduced KV state back; mask + build Ksum_bcast."""
                kvbd, ksb = [], []
                for ci in (0, 1):
                    t = callp.tile([128, 257], F16, tag=f"kvar{ci}", name=f"kvar{ci}")
                    nc.sync.dma_start(out=t[:], in_=arout[ci])
                    bd = callp.tile([128, 128], F16, tag=f"kvbd{ci}", name=f"kvbd{ci}")
                    nc.vector.tensor_tensor(bd[:], t[:, ci * 128:(ci + 1) * 128],
                                            mask[:], OP.mult)
                    ks32 = callp.tile([128, 1], F32, tag=f"ks32{ci}",
                                      name=f"ks32{ci}")
                    nc.vector.tensor_copy(ks32[:], t[:, 256:257])
                    kb = callp.tile([128, 128], F16, tag=f"ksb{ci}", name=f"ksb{ci}")
                    nc.vector.tensor_scalar(kb[:], mask[:], ks32[:], None,
                                            OP.mult)
                    kvbd.append(bd)
                    ksb.append(kb)
                return kvbd, ksb

            class QPhi:
                """Q projection + phi, tile-streamable (AR-independent).
                Emitted either standalone or interleaved into a running
                loop2 via its per-tile callback, keeping the PE fed across
                the loop boundary."""

                def __init__(self, f, w):
                    self.f, self.w = f, w
                    self.qphi_all = []
                    self.done = 0

                def emit(self, upto):
                    f, w = self.f, self.w
                    for it in range(self.done, min(upto, NTT)):
                        ts = slice(it * TT, it * TT + TT)
                        pq_ci, qp_ci = [], []
                        for ci in (0, 1):
                            pq = psump.tile([128, TT], F32, tag="ring", bufs=6,
                                            name="pq")
                            for cj in (0, 1):
                                nc.tensor.matmul(
                                    pq[:],
                                    w["wq"][cj][:, ci * 128:(ci + 1) * 128],
                                    x[(f, cj)][:, ts],
                                    start=(cj == 0), stop=(cj == 1))
                            pq_ci.append(pq)
                        for ci in (0, 1):
                            mq = p2s.tile([128, TT], F16, tag=f"mq{ci}", bufs=4,
                                          name="mq")
                            nc.vector.tensor_scalar_min(mq[:], pq_ci[ci][:], 0.0)
                            _act(nc, mq[:], mq[:], AF.Exp)
                            qp = p2s.tile([128, TT], F16, tag=f"qphi{ci}",
                                          bufs=7, name="qp")
                            nc.vector.scalar_tensor_tensor(
                                qp[:], pq_ci[ci][:], 0.0, mq[:], OP.max, OP.add)
                            qp_ci.append(qp)
                        self.qphi_all.append(qp_ci)
                    self.done = max(self.done, min(upto, NTT))

                def finish(self):
                    self.emit(NTT)
                    return self.qphi_all

            def loop2(f, w, kvbd, ksb, qphi_all, after_tile=None):
                """Z, msg, merge, LN1(var-only), MLP, LN2(var-only), residual.
                STAGE-MAJOR: each stage is emitted for all NTT tiles before
                the next stage, so every in-order engine queue always holds
                independent work from other tiles and never head-of-line
                blocks on a cross-engine round trip."""
                tss = [slice(it * TT, it * TT + TT) for it in range(NTT)]
                # S1: Zden matmuls + reciprocal via exp(-ln)
                zr_all = []
                for it in range(NTT):
                    zr = []
                    for ci in (0, 1):
                        pz = psump.tile([128, TT], F32, tag="ring", bufs=6,
                                        name="pz")
                        nc.tensor.matmul(pz[:], ksb[ci][:], qphi_all[it][ci][:],
                                         start=True, stop=True)
                        zl = p2s.tile([128, TT], F32, tag=f"zl{ci}", bufs=2, name="zl")
                        _act(nc, zl[:], pz[:], AF.Ln)
                        z = p2s.tile([128, TT], F16, tag=f"zr{ci}", bufs=6,
                                     name="zr")
                        _act(nc, z[:], zl[:], AF.Exp, scale=-1.0)
                        zr.append(z)
                    zr_all.append(zr)
                # S2: msg matmuls + Z scaling
                msgz_all = []
                for it in range(NTT):
                    msgz = []
                    for ci in (0, 1):
                        pm = psump.tile([128, TT], F32, tag="ring", bufs=6,
                                        name="pm")
                        nc.tensor.matmul(pm[:], kvbd[ci][:], qphi_all[it][ci][:],
                                         start=True, stop=True)
                        mz = p2s.tile([128, TT], F16, tag=f"msgz{ci}", bufs=6,
                                      name="mz")
                        nc.vector.tensor_tensor(mz[:], pm[:], zr_all[it][ci][:],
                                                OP.mult)
                        msgz.append(mz)
                    msgz_all.append(msgz)
                # S3: merge (centered Wm) + evict + square
                yc_all, y2_all = [], []
                for it in range(NTT):
                    yc, y2 = [], []
                    for mo in (0, 1):
                        pmg = psump.tile([128, TT], F32, tag="ring", bufs=6,
                                         name="pmg")
                        for ci in (0, 1):
                            nc.tensor.matmul(pmg[:],
                                             w["wm"][ci][:, mo * 128:(mo + 1) * 128],
                                             msgz_all[it][ci][:],
                                             start=(ci == 0), stop=(ci == 1))
                        t = p2s.tile([128, TT], F16, tag=f"yc{mo}", bufs=6,
                                     name="yc")
                        nc.scalar.copy(t[:], pmg[:])
                        yc.append(t)
                        t2 = p2s.tile([128, TT], F16, tag=f"y2{mo}", bufs=3,
                                      name="y2")
                        if mo == 0:
                            nc.vector.tensor_tensor(t2[:], t[:], t[:], OP.mult)
                        else:
                            nc.gpsimd.tensor_tensor(t2[:], t[:], t[:], OP.mult)
                        y2.append(t2)
                    yc_all.append(yc)
                    y2_all.append(y2)
                # S4: var matmul + rsqrt via exp(-0.5 ln)
                rsd_all = []
                for it in range(NTT):
                    pvar = psump.tile([128, TT], F32, tag="ring", bufs=6,
                                      name="pvar")
                    for mo in (0, 1):
                        nc.tensor.matmul(pvar[:], ones[:], y2_all[it][mo][:],
                                         start=(mo == 0), stop=(mo == 1))
                    tv = p2s.tile([128, TT], F32, tag="tv", bufs=2, name="tv")
                    _act(nc, tv[:], pvar[:], AF.Ln, bias=epsln[:, 0:1])
                    rsd = p2s.tile([128, TT], F16, tag="rsd", bufs=6, name="rsd")
                    _act(nc, rsd[:], tv[:], AF.Exp, scale=-0.5)
                    rsd_all.append(rsd)
                # S5: msghat (f16 x f16 on Pool; DVE is the scarcer engine)
                mh_all = []
                for it in range(NTT):
                    mh = []
                    for mo in (0, 1):
                        t = p2s.tile([128, TT], F16, tag=f"mh{mo}", bufs=6,
                                     name="mh")
                        if mo == 0:
                            nc.vector.tensor_tensor(t[:], yc_all[it][mo][:],
                                                    rsd_all[it][:], OP.mult)
                        else:
                            nc.gpsimd.tensor_tensor(t[:], yc_all[it][mo][:],
                                                    rsd_all[it][:], OP.mult)
                        mh.append(t)
                    mh_all.append(mh)
                # S6: W1 + fused bias+relu
                r1_all = []
                for it in range(NTT):
                    r1 = []
                    for mo in range(4):
                        pw1 = psump.tile([128, TT], F32, tag="ring", bufs=6,
                                         name="pw1")
                        for cj in range(4):
                            rhs = (x[(f, cj)][:, tss[it]] if cj < 2
                                   else mh_all[it][cj - 2][:])
                            nc.tensor.matmul(pw1[:],
                                             w["w1"][cj][:, mo * 128:(mo + 1) * 128],
                                             rhs, start=(cj == 0), stop=(cj == 3))
                        t = p2s.tile([128, TT], F16, tag=f"r1{mo}", bufs=6,
                                     name="r1")
                        if mo < 2:
                            _act(nc, t[:], pw1[:], AF.Relu,
                                 bias=w["b1"][:, mo:mo + 1])
                        else:
                            nc.vector.tensor_scalar(t[:], pw1[:],
                                                    w["b1"][:, mo:mo + 1], 0.0,
                                                    OP.add, OP.max)
                        r1.append(t)
                    r1_all.append(r1)
                # S7: W2 (centered) + evict + square
                yd_all, yd2_all = [], []
                for it in range(NTT):
                    yd, yd2 = [], []
                    for mo in (0, 1):
                        pw2 = psump.tile([128, TT], F32, tag="ring", bufs=6,
                                         name="pw2")
                        for cj in range(4):
                            nc.tensor.matmul(pw2[:],
                                             w["w2"][cj][:, mo * 128:(mo + 1) * 128],
                                             r1_all[it][cj][:], start=(cj == 0),
                                             stop=(cj == 3))
                        t = p2s.tile([128, TT], F16, tag=f"yd{mo}", bufs=6,
                                     name="yd")
                        nc.vector.tensor_copy(t[:], pw2[:])
                        yd.append(t)
                        t2 = p2s.tile([128, TT], F16, tag=f"yd2{mo}", bufs=3,
                                      name="yd2")
                        if mo == 0:
                            nc.vector.tensor_tensor(t2[:], t[:], t[:], OP.mult)
                        else:
                            nc.gpsimd.tensor_tensor(t2[:], t[:], t[:], OP.mult)
                        yd2.append(t2)
                    yd_all.append(yd)
                    yd2_all.append(yd2)
                # S8: var2 + rsqrt
                rsd2_all = []
                for it in range(NTT):
                    pvar2 = psump.tile([128, TT], F32, tag="ring", bufs=6,
                                       name="pvar2")
                    for mo in (0, 1):
                        nc.tensor.matmul(pvar2[:], ones[:], yd2_all[it][mo][:],
                                         start=(mo == 0), stop=(mo == 1))
                    tv2 = p2s.tile([128, TT], F32, tag="tv2", bufs=2, name="tv2")
                    _act(nc, tv2[:], pvar2[:], AF.Ln, bias=epsln[:, 0:1])
                    rsd2 = p2s.tile([128, TT], F16, tag="rsd2", bufs=6,
                                    name="rsd2")
                    _act(nc, rsd2[:], tv2[:], AF.Exp, scale=-0.5)
                    rsd2_all.append(rsd2)
                # S9: LN2 affine + residual update. fast_ln2 (ln2 affine is
                # identity, true for this model): dl = yd*rsd2 on DVE, the
                # residual add on Pool.
                for it in range(NTT):
                    for mo in (0, 1):
                        dl = p2s.tile([128, TT], F16, tag=f"dl{mo}", bufs=2, name="dl")
                        if fast_ln2:
                            nc.vector.tensor_tensor(
                                dl[:], yd_all[it][mo][:], rsd2_all[it][:],
                                OP.mult)
                            nc.gpsimd.tensor_tensor(
                                x[(f, mo)][:, tss[it]], x[(f, mo)][:, tss[it]],
                                dl[:], OP.add)
                        else:
                            nc.vector.scalar_tensor_tensor(
                                dl[:], yd_all[it][mo][:], w["l2w"][:, mo:mo + 1],
                                rsd2_all[it][:], OP.mult, OP.mult)
                            nc.vector.scalar_tensor_tensor(
                                x[(f, mo)][:, tss[it]], dl[:],
                                w["l2b"][:, mo:mo + 1],
                                x[(f, mo)][:, tss[it]], OP.add, OP.add)
                    if after_tile is not None:
                        after_tile(it)

            def load_weights(li):
                w = {}
                w["wkv"] = []
                for ci in (0, 1):
                    t = wp.tile([128, 512], F16, tag=f"wkv{ci}", name=f"wkv{ci}")
                    nc.sync.dma_start(
                        out=t[:], in_=wkv_d[li, ci * 128:(ci + 1) * 128, :])
                    w["wkv"].append(t)
                for nm, dram in (("wq", wq_d), ("wm", wm_d)):
                    tiles = []
                    for ci in (0, 1):
                        t = wp.tile([128, 256], F16, tag=f"{nm}{ci}",
                                    name=f"{nm}{ci}")
                        nc.sync.dma_start(
                            out=t[:], in_=dram[li, ci * 128:(ci + 1) * 128, :])
                        tiles.append(t)
                    w[nm] = tiles
                w["w1"] = []
                for ci in range(4):
                    t = wp.tile([128, 512], F16, tag=f"w1{ci}", name=f"w1{ci}")
                    nc.sync.dma_start(
                        out=t[:], in_=w1_d[li, ci * 128:(ci + 1) * 128, :])
                    w["w1"].append(t)
                w["w2"] = []
                for ci in range(4):
                    t = wp.tile([128, 256], F16, tag=f"w2{ci}", name=f"w2{ci}")
                    nc.sync.dma_start(
                        out=t[:], in_=w2_d[li, ci * 128:(ci + 1) * 128, :])
                    w["w2"].append(t)
                for nm, dram, nf in (("b1", b1_d, 4), ("l2w", l2w_d, 2),
                                     ("l2b", l2b_d, 2)):
                    t = wp.tile([128, nf], F32, tag=nm, name=nm)
                    nc.sync.dma_start(out=t[:], in_=dram[li])
                    w[nm] = t
                return w

            # per-tile callback: stream pending Phase1 token blocks (paced by
            # the x tiles they read) and pending QPhi tiles into the running
            # loop2 so the PE stays fed across loop/layer boundaries.
            def interleave_cb(p1, qp):
                if p1 is None and qp is None:
                    return None
                def cb(it):
                    if p1 is not None:
                        p1.blocks(int((it + 1) * TT / 128))
                    if qp is not None:
                        qp.emit(it + 1)
                return cb

            # schedule
            LAYER_NAMES = ['self', 'cross'] * (n_layers // 2 + 1)
            ws = [load_weights(0)]
            pending = None   # Phase1 for the next consumer, streamed early
            pend_qp = None   # QPhi for feature 0 of the next layer

            for li in range(n_layers):
                w = ws[li]
                if li + 1 < n_layers:
                    ws.append(load_weights(li + 1))
                nxt = ws[li + 1] if li + 1 < n_layers else None
                if LAYER_NAMES[li] == 'self':
                    if pending is None:
                        p1a = Phase1(0, w)
                        p1a.blocks(NKT)
                    else:
                        p1a = pending
                    ar0 = p1a.finish()
                    p1b = Phase1(1, w)
                    p1b.blocks(NKT)
                    ar1 = p1b.finish()
                    qp0 = pend_qp if pend_qp is not None else QPhi(0, w)
                    qphi0 = qp0.finish()
                    kvbd0, ksb0 = finish_kv(ar0)
                    qp1 = QPhi(1, w)
                    loop2(0, w, kvbd0, ksb0, qphi0)
                    qphi1 = qp1.finish()
                    kvbd1, ksb1 = finish_kv(ar1)
                    # next layer is cross: P1(f0, src=f1) + QPhi(f0, next)
                    pending = Phase1(1, nxt) if nxt is not None else None
                    pend_qp = None
                    loop2(1, w, kvbd1, ksb1, qphi1,
                          interleave_cb(pending, None))
                else:
                    if pending is None:
                        p1a = Phase1(1, w)
                        p1a.blocks(NKT)
                    else:
                        p1a = pending
                    ar0 = p1a.finish()
                    qp0 = pend_qp if pend_qp is not None else QPhi(0, w)
                    qphi0 = qp0.finish()
                    kvbd0, ksb0 = finish_kv(ar0)
                    # P1(f1, src=f0) + QPhi(f1, this layer) stream off loop2(0)
                    p1b = Phase1(0, w)
                    qp1 = QPhi(1, w)
                    loop2(0, w, kvbd0, ksb0, qphi0, interleave_cb(p1b, None))
                    ar1 = p1b.finish()
                    qphi1 = qp1.finish()
                    kvbd1, ksb1 = finish_kv(ar1)
                    # next layer is self: P1(f0, src=f0) + QPhi(f0, next)
                    pending = Phase1(0, nxt) if nxt is not None else None
                    pend_qp = None
                    loop2(1, w, kvbd1, ksb1, qphi1,
                          interleave_cb(pending, None))

            for f in (0, 1):
                for ci in (0, 1):
                    nc.sync.dma_start(out=yout[f][ci * 128:(ci + 1) * 128, :],
                                      in_=x[(f, ci)][:])

    split_multi_waits(nc)
    return nc


def prep_inputs(inputs, n_layers=8):
    """Host-side: shard features, fold ln1 into W1/bias1, column-center
    Wm and W2 (folds the LayerNorm mean-subtract into the matmul), pack
    constants. Returns in_maps for the 8 cores."""
    f32 = np.float32
    feat0, feat1 = np.asarray(inputs["feat0"]), np.asarray(inputs["feat1"])
    Wq, Wk, Wv, Wm = (np.asarray(inputs[k], dtype=f32)
                      for k in ("Wq", "Wk", "Wv", "Wm"))
    W1, W2 = np.asarray(inputs["W1"], dtype=f32), np.asarray(inputs["W2"], dtype=f32)
    ln1_w, ln1_b = np.asarray(inputs["ln1_w"], dtype=f32), np.asarray(inputs["ln1_b"], dtype=f32)
    ln2_w, ln2_b = np.asarray(inputs["ln2_w"], dtype=f32), np.asarray(inputs["ln2_b"], dtype=f32)

    W1eff = W1[:n_layers].copy()
    W1eff[:, C:, :] *= ln1_w[:n_layers, :, None]
    b1 = np.einsum("lc,lcd->ld", ln1_b[:n_layers], W1[:n_layers, C:, :])
    b1p = np.ascontiguousarray(b1.reshape(n_layers, 4, 128).transpose(0, 2, 1))
    l2wp = np.ascontiguousarray(ln2_w[:n_layers].reshape(n_layers, 2, 128).transpose(0, 2, 1))
    l2bp = np.ascontiguousarray(ln2_b[:n_layers].reshape(n_layers, 2, 128).transpose(0, 2, 1))

    # column-center Wm and W2: out @ (W - rowmean) is exactly mean-centered
    WmC = Wm[:n_layers] - Wm[:n_layers].mean(axis=2, keepdims=True)
    W2C = W2[:n_layers] - W2[:n_layers].mean(axis=2, keepdims=True)
    # fused K|V projection weight [C, 2C] = [Wk | Wv]
    Wkv = np.concatenate([Wk[:n_layers], Wv[:n_layers]], axis=2)

    f16 = np.float16
    idx = np.arange(128)
    # plain 0/1 mask: the 1/64 range scale is applied before the AllReduce
    blockmask = (idx[:, None] // 32 == idx[None, :] // 32).astype(f16)
    onesC = np.full((128, 128), 1.0 / C, dtype=f16)

    shared = {
        "Wkv": np.ascontiguousarray(Wkv).astype(f16),
        "Wq": np.ascontiguousarray(Wq[:n_layers]).astype(f16),
        "WmC": np.ascontiguousarray(WmC).astype(f16),
        "W1": np.ascontiguousarray(W1eff).astype(f16),
        "W2C": np.ascontiguousarray(W2C).astype(f16),
        "b1p": b1p, "l2wp": l2wp, "l2bp": l2bp,
        "blockmask": blockmask, "onesC": onesC,
    }
    in_maps = []
    for c in range(N_CORES):
        b, h = c // 2, c % 2
        rows = slice(h * R, (h + 1) * R)
        m = dict(shared)
        m["xT0"] = np.ascontiguousarray(feat0[b, rows].T).astype(f16)
        m["xT1"] = np.ascontiguousarray(feat1[b, rows].T).astype(f16)
        in_maps.append(m)
    return in_maps


def assemble_outputs(results):
    feat0 = np.empty((B, L, C), np.float32)
    feat1 = np.empty((B, L, C), np.float32)
    for c in range(N_CORES):
        b, h = c // 2, c % 2
        rows = slice(h * R, (h + 1) * R)
        feat0[b, rows] = results[c]["yT0"].T.astype(np.float32)
        feat1[b, rows] = results[c]["yT1"].T.astype(np.float32)
    return feat0, feat1


_cache = {}


def get_nc(n_layers=8, fast_ln2=True):
    key = (n_layers, fast_ln2)
    if key not in _cache:
        _cache[key] = build(n_layers, fast_ln2)
    return _cache[key]


def kernel(**inputs):
    from concourse.bass_utils import run_bass_kernel_spmd
    fast = (np.all(np.asarray(inputs["ln2_w"]) == 1.0)
            and np.all(np.asarray(inputs["ln2_b"]) == 0.0))
    nc = get_nc(8, fast)
    in_maps = prep_inputs(inputs, 8)
    res = run_bass_kernel_spmd(nc, in_maps, list(range(N_CORES)))
    return assemble_outputs(res.results)
